# revision 20
# baseline (speedup 1.0000x reference)
"""Trainium2 Bass kernel for CustomConv1d.

Problem: y = conv1d(x, weight, bias), x [32, 256, 4096] f32,
weight [256, 256, 5] f32, bias [256] f32, stride 1, pad 2.

Strategy: data-parallel over batch across 8 NeuronCores (4 batches/core,
weights+bias broadcast, no collectives). Per core the conv is computed as
matmuls on the tensor engine: for each output-channel chunk (128) and each
512-wide output tile, accumulate 10 matmuls in PSUM (5 taps x 2 input-channel
chunks of 128):

  out[co, w] = sum_{k, ci} weight[co, ci, k] * xpad[ci, w + k]

with lhsT = weight slice [ci(128 part), co(128)] and rhs = x slice
[ci(128 part), 512]. Operands are cast to bf16 on the host (~2.3e-3 l2 rel
err, well under the 2e-2 gate); PSUM accumulates fp32 and the output is
stored fp32. bf16 halves the x/weight HBM traffic and avoids the fp32-HIGH
PE power-throttle (fp32 matmul cadence is ~233 ns/512 rows vs 216 for bf16).
x arrives host-padded ([.., W+4]) so every tap is a plain contiguous slice.

DMA reality (measured): the first DMA byte moves ~9 us in (engine preambles
~6-7 us + trigger + DGE latency); per-transfer throughput scales with the
per-partition descriptor line size (1 KB lines supply only ~130 GB/s
aggregate, 4-16 KB lines reach 220+ GB/s); concurrent transfers share
bandwidth per-transfer, not per-byte; and the Tile scheduler is a greedy
list scheduler, so a DMA with no dependencies starts at t=0 regardless of
emission order. Hence:
 - weights are host-packed per output-channel half so the first matmul
   gates on one 327 KB fat-line (2560 B) transfer, triggered first on Sync;
 - batch 0's first two width-groups ride small dedicated tiles, the rest of
   batch 0 on 2-group halo chunks, all ungated (they are the consumption
   frontier);
 - batches 1-3 load as half-row tiles (4104 B lines) gated behind batch 0's
   last chunk via dummy-tile aliasing (gpsimd writes a gate tile only after
   that chunk lands; each later load aliases a gate/predecessor buffer, so
   the WAR/WAW edge defers it) — otherwise they'd flood HBM at t=0 and
   starve the startup-critical loads;
 - output stores trigger from the otherwise-idle Scalar engine.

The tail is trimmed by splitting the last group's bias-add + store into
256-col pieces so the final store isn't waiting on a full 512-col drain.
"""

import os

import numpy as np
import ml_dtypes

import concourse.mybir as mybir
import concourse.tile as tile
from concourse import bacc
from concourse.bass_utils import run_bass_kernel_spmd


BF16 = mybir.dt.bfloat16
F32 = mybir.dt.float32

B, CIN, COUT, W, K, PAD = 32, 256, 256, 4096, 5, 2
NCORES = 8
BPC = B // NCORES          # batches per core
P = 128                    # partition dim
NT = 512                   # moving-operand tile (one fp32 PSUM bank)
N_CIC = CIN // P           # input-channel chunks
N_COC = COUT // P          # output-channel chunks
N_WT = W // NT             # output width tiles
WPADDED = W + 2 * PAD
ST = 2 * NT                # output store chunk (overlap tail stores)
CWH = 2 * NT + 2 * PAD     # halo chunk: 2 groups + taps
NTF = NT + 2 * PAD         # fine first-chunk: 1 group + taps
HALF = W // 2 + 2 * PAD    # half-row tile for batches 1..3
NWARM = 5                  # PE warm-up matmuls (bridge preamble->first data)


def _build_program():
    # Bacc (not plain Bass): its finalize() runs generate_event_semaphores,
    # which splits multi-sem waits into event-semaphore chains — the TRN2
    # walrus here accepts at most one sync wait per regular instruction.
    nc = bacc.Bacc()
    # x arrives host-padded: x[b, ci, :] = [0, 0, x_orig, 0, 0] (WPADDED cols)
    x_d = nc.declare_dram_parameter("x", [BPC, CIN, WPADDED], BF16, isOutput=False)
    # batch 0's first group repacked head-blob-style: [ci, cic, col] gives
    # 2064 B contiguous per partition, one fat-line startup transfer instead
    # of two thin ones (thin 1 KB lines supply only ~130 GB/s)
    x0h_d = nc.declare_dram_parameter("x0h", [P, N_CIC, NTF], BF16, isOutput=False)
    # weights host-packed as wt[ci, coc, k, cic, co]: each coc half is one
    # contiguous-line (2560 B/partition) transfer
    wt_d = nc.declare_dram_parameter(
        "wt", [P, N_COC, N_CIC, K, P], BF16, isOutput=False
    )
    b_d = nc.declare_dram_parameter("bias2", [P, N_COC], F32, isOutput=False)
    o_d = nc.declare_dram_parameter("out", [BPC, COUT, W], F32, isOutput=True)

    with tile.TileContext(nc) as tc:
        with (
            tc.tile_pool(name="wpool", bufs=1) as wpool,
            tc.tile_pool(name="xpool", bufs=4) as xpool,
            tc.tile_pool(name="opool", bufs=2 * N_COC) as opool,
            tc.tile_pool(name="psum", bufs=8, space="PSUM") as pspool,
        ):
            warm = wpool.tile([P, NT], BF16)
            nc.vector.memset(warm[:], 0.0)

            # Startup-critical loads, Sync trigger queue, consumption order.
            w_co = [[None] * N_CIC for _ in range(N_COC)]
            for coc in range(N_COC):
                for cic in range(N_CIC):
                    w_co[coc][cic] = wpool.tile(
                        [P, K, P], BF16, name=f"w_co{coc}_{cic}"
                    )
            x0h = [None] * N_CIC
            for cic in range(N_CIC):
                x0h[cic] = xpool.tile([P, NTF], BF16, tag="x0h", bufs=2,
                                      name=f"x0h_{cic}")
            xf1 = []
            for cic in range(N_CIC):
                xf1.append(xpool.tile(
                    [P, NTF], BF16, tag="xf", bufs=2, name=f"xf1_{cic}"
                ))
            # just-in-time startup chain: each link ~300 KB, consumed in
            # ~1.1 us of matmuls while the next link streams in
            nc.sync.dma_start(w_co[0][0][:], wt_d[:, 0, 0])
            nc.sync.dma_start(x0h[0][:], x0h_d[:, 0])
            nc.sync.dma_start(w_co[0][1][:], wt_d[:, 0, 1])
            nc.sync.dma_start(x0h[1][:], x0h_d[:, 1])
            for cic in range(N_CIC):
                nc.sync.dma_start(
                    xf1[cic][:], x_d[0, cic * P:(cic + 1) * P, NT:NT + NTF]
                )

            # Rest of batch 0: 2-group halo chunks (chunk c covers padded
            # cols [1024c, 1024c + CWH), read by groups n=2c, 2c+1).
            # c1 loads immediately; c2/c3 are staged behind c1/c2's arrival
            # via gate dummies so the first-group-critical transfers get the
            # early bandwidth.
            scratch = wpool.tile([P, 1], BF16)
            xc = {}  # (c, cic) -> tile, c in 1..3
            for cic in range(N_CIC):
                t = xpool.tile([P, CWH], BF16, tag="xc1", bufs=N_CIC,
                               name=f"x0_1_{cic}")
                xc[(1, cic)] = t
                nc.sync.dma_start(
                    t[:], x_d[0, cic * P:(cic + 1) * P, 2 * NT:2 * NT + CWH]
                )
            b_sb = wpool.tile([P, N_COC], F32)
            nc.sync.dma_start(b_sb[:], b_d[:])
            # ring layout (bufs=4): [gc2 x2, gc3 x2, c2 x2 (alias gc2),
            # c3 x2 (alias gc3)] — each chunk's DMA waits only its gate,
            # whose writer waits the previous chunk's arrival
            gdum = {}
            for c in range(2, N_WT // 2):
                for cic in range(N_CIC):
                    gdum[(c, cic)] = xpool.tile(
                        [P, CWH], BF16, tag="xc", bufs=4, name=f"gc{c}_{cic}"
                    )
            for c in range(2, N_WT // 2):
                gate_src = xc[(c - 1, N_CIC - 1)]
                for cic in range(N_CIC):
                    dummy = gdum[(c, cic)]
                    nc.gpsimd.tensor_scalar_add(
                        dummy[:, 0:1], gate_src[:, 0:1], 0.0
                    )
                    nc.gpsimd.tensor_scalar_add(scratch[:], dummy[:, 0:1], 0.0)
                for cic in range(N_CIC):
                    t = xpool.tile([P, CWH], BF16, tag="xc", bufs=4,
                                   name=f"x0_{c}_{cic}")
                    xc[(c, cic)] = t
                    nc.sync.dma_start(
                        t[:],
                        x_d[0, cic * P:(cic + 1) * P,
                            c * 2 * NT:c * 2 * NT + CWH],
                    )
            # second weight half: triggers sit after c3's in the Sync
            # queue, keeping them out of the startup-critical window
            # (needed ~17 us later than w_co[0])
            for cic in range(N_CIC):
                nc.sync.dma_start(w_co[1][cic][:], wt_d[:, 1, cic])

            # Batches 1..3: half-row tiles deferred by gate aliasing. The 4
            # gate dummies are written (gpsimd) only once batch 0's last
            # chunk has landed; batch b+1's tiles alias batch b's (same ring
            # position), so each load starts once its predecessor half is
            # fully consumed — mid-way through the previous batch.
            last_c3 = xc[(N_WT // 2 - 1, N_CIC - 1)]
            for i in range(4):
                dummy = xpool.tile([P, HALF], BF16, tag="xh", bufs=4,
                                   name=f"gate{i}")
                nc.gpsimd.tensor_scalar_add(dummy[:, 0:1], last_c3[:, 0:1], 0.0)
                nc.gpsimd.tensor_scalar_add(scratch[:], dummy[:, 0:1], 0.0)
            xh = {}  # (b, h, cic) -> tile; h half covers cols [h*2048, +HALF)
            for b in range(1, BPC):
                for h in range(2):
                    for cic in range(N_CIC):
                        t = xpool.tile([P, HALF], BF16, tag="xh", bufs=4,
                                       name=f"x{b}_{h}_{cic}")
                        xh[(b, h, cic)] = t
                        nc.sync.dma_start(
                            t[:],
                            x_d[b, cic * P:(cic + 1) * P,
                                h * (W // 2):h * (W // 2) + HALF],
                        )

            ps_warm = pspool.tile([P, NT], F32, tag="ps", name="ps_warm")
            for _ in range(NWARM):
                nc.tensor.matmul(ps_warm[:], warm[:, 0:P], warm[:])

            for b in range(BPC):
                last_pass = b == BPC - 1
                for coc in range(N_COC):
                    last_coc = last_pass and coc == N_COC - 1
                    ot = opool.tile([P, W], F32, tag="o")
                    st = NT if last_coc else ST
                    for n in range(N_WT):
                        ps = pspool.tile([P, NT], F32, tag="ps", name=f"ps{b}_{coc}_{n}")
                        idx = 0
                        # cic-major: the first K matmuls of batch 0's first
                        # group need only the cic0 tile, giving cic1's DMA
                        # an extra ~1 us — a sub-300ns stall here resets the
                        # PE pstate ramp and costs ~2.5 us of half-speed
                        for cic in range(N_CIC):
                            for k in range(K):
                                if b == 0:
                                    if n == 0:
                                        rhs = x0h[cic][:, k:k + NT]
                                    elif n == 1:
                                        rhs = xf1[cic][:, k:k + NT]
                                    else:
                                        rhs = xc[(n // 2, cic)][
                                            :, (n % 2) * NT + k:(n % 2) * NT + k + NT
                                        ]
                                else:
                                    h, nh = divmod(n, N_WT // 2)
                                    rhs = xh[(b, h, cic)][
                                        :, nh * NT + k:nh * NT + k + NT
                                    ]
                                nc.tensor.matmul(
                                    ps[:],
                                    w_co[coc][cic][:, k, :],
                                    rhs,
                                    start=(idx == 0),
                                    stop=(idx == K * N_CIC - 1),
                                )
                                idx += 1
                        if last_coc and n == N_WT - 1:
                            # final group: drain the two 256-col halves on
                            # DIFFERENT engines (vector + scalar activation
                            # Identity = in*1 + bias) and trigger the stores
                            # from different queues, so the kernel tail is
                            # one quarter-size drain + one small store deep
                            lo = n * NT
                            hf = NT // 2
                            nc.vector.tensor_scalar_add(
                                ot[:, lo:lo + hf], ps[:, 0:hf],
                                b_sb[:, coc:coc + 1],
                            )
                            nc.sync.dma_start(
                                o_d[b, coc * P:(coc + 1) * P, lo:lo + hf],
                                ot[:, lo:lo + hf],
                            )
                            nc.scalar.activation(
                                ot[:, lo + hf:lo + NT], ps[:, hf:NT],
                                mybir.ActivationFunctionType.Identity,
                                bias=b_sb[:, coc:coc + 1],
                            )
                            nc.scalar.dma_start(
                                o_d[b, coc * P:(coc + 1) * P,
                                    lo + hf:lo + NT],
                                ot[:, lo + hf:lo + NT],
                            )
                        else:
                            nc.vector.tensor_scalar_add(
                                ot[:, n * NT:(n + 1) * NT], ps[:],
                                b_sb[:, coc:coc + 1],
                            )
                            # store as soon as a full chunk of st cols is
                            # ready (trigger from the Scalar engine; Sync
                            # paces the x loads)
                            if ((n + 1) * NT) % st == 0:
                                c0 = (n + 1) * NT - st
                                nc.scalar.dma_start(
                                    o_d[b, coc * P:(coc + 1) * P, c0:c0 + st],
                                    ot[:, c0:c0 + st],
                                )
    nc.finalize()
    return nc


_NC_CACHE = []


def kernel(x, weight, bias):
    assert x.shape == (B, CIN, W) and weight.shape == (COUT, CIN, K)
    if not _NC_CACHE:
        _NC_CACHE.append(_build_program())
    nc = _NC_CACHE[0]

    # wt[ci, coc, cic, k, co] = weight[coc*128+co, cic*128+ci, k]
    wt = np.ascontiguousarray(
        weight.astype(np.float32)
        .reshape(N_COC, P, N_CIC, P, K)   # [coc, co, cic, ci, k]
        .transpose(3, 0, 2, 4, 1)         # [ci, coc, cic, k, co]
    ).astype(ml_dtypes.bfloat16)
    bias2 = np.ascontiguousarray(bias.astype(np.float32).reshape(N_COC, P).T)
    xpad = np.pad(x.astype(np.float32), ((0, 0), (0, 0), (PAD, PAD))).astype(
        ml_dtypes.bfloat16
    )
    in_maps = [
        {
            "x": np.ascontiguousarray(xpad[i * BPC:(i + 1) * BPC]),
            # head blob: x0h[ci, cic, col] = xpad[first batch, cic*128+ci, col]
            "x0h": np.ascontiguousarray(
                xpad[i * BPC, :, :NTF].reshape(N_CIC, P, NTF).transpose(1, 0, 2)
            ),
            "wt": wt,
            "bias2": bias2,
        }
        for i in range(NCORES)
    ]
    res = run_bass_kernel_spmd(
        nc,
        in_maps,
        list(range(NCORES)),
        trace=bool(int(os.environ.get("KERNEL_TRACE", "0"))),
    )
    kernel.last_results = res
    return np.concatenate([res.results[i]["out"] for i in range(NCORES)], axis=0)


# revision 21
# speedup vs baseline: 1.0197x; 1.0197x over previous
"""Trainium2 Bass kernel for CustomConv1d.

Problem: y = conv1d(x, weight, bias), x [32, 256, 4096] f32,
weight [256, 256, 5] f32, bias [256] f32, stride 1, pad 2.

Strategy: data-parallel over batch across 8 NeuronCores (4 batches/core,
weights+bias broadcast, no collectives). Per core the conv is computed as
matmuls on the tensor engine: for each output-channel chunk (128) and each
512-wide output tile, accumulate 10 matmuls in PSUM (5 taps x 2 input-channel
chunks of 128):

  out[co, w] = sum_{k, ci} weight[co, ci, k] * xpad[ci, w + k]

with lhsT = weight slice [ci(128 part), co(128)] and rhs = x slice
[ci(128 part), 512]. Operands are cast to bf16 on the host (~2.3e-3 l2 rel
err, well under the 2e-2 gate); PSUM accumulates fp32 and the output is
stored fp32. bf16 halves the x/weight HBM traffic and avoids the fp32-HIGH
PE power-throttle (fp32 matmul cadence is ~233 ns/512 rows vs 216 for bf16).
x arrives host-padded ([.., W+4]) so every tap is a plain contiguous slice.

DMA reality (measured): the first DMA byte moves ~9 us in (engine preambles
~6-7 us + trigger + DGE latency); per-transfer throughput scales with the
per-partition descriptor line size (1 KB lines supply only ~130 GB/s
aggregate, 4-16 KB lines reach 220+ GB/s); concurrent transfers share
bandwidth per-transfer, not per-byte; and the Tile scheduler is a greedy
list scheduler, so a DMA with no dependencies starts at t=0 regardless of
emission order. Hence:
 - weights are host-packed per output-channel half so the first matmul
   gates on one 327 KB fat-line (2560 B) transfer, triggered first on Sync;
 - batch 0's first two width-groups ride small dedicated tiles, the rest of
   batch 0 on 2-group halo chunks, all ungated (they are the consumption
   frontier);
 - batches 1-3 load as half-row tiles (4104 B lines) gated behind batch 0's
   last chunk via dummy-tile aliasing (gpsimd writes a gate tile only after
   that chunk lands; each later load aliases a gate/predecessor buffer, so
   the WAR/WAW edge defers it) — otherwise they'd flood HBM at t=0 and
   starve the startup-critical loads;
 - output stores trigger from the otherwise-idle Scalar engine.

The tail is trimmed by splitting the last group's bias-add + store into
256-col pieces so the final store isn't waiting on a full 512-col drain.
"""

import os

import numpy as np
import ml_dtypes

import concourse.mybir as mybir
import concourse.tile as tile
from concourse import bacc
from concourse.bass_utils import run_bass_kernel_spmd


BF16 = mybir.dt.bfloat16
F32 = mybir.dt.float32

B, CIN, COUT, W, K, PAD = 32, 256, 256, 4096, 5, 2
NCORES = 8
BPC = B // NCORES          # batches per core
P = 128                    # partition dim
NT = 512                   # moving-operand tile (one fp32 PSUM bank)
N_CIC = CIN // P           # input-channel chunks
N_COC = COUT // P          # output-channel chunks
N_WT = W // NT             # output width tiles
WPADDED = W + 2 * PAD
ST = 2 * NT                # output store chunk (overlap tail stores)
CWH = 2 * NT + 2 * PAD     # halo chunk: 2 groups + taps
NTF = NT + 2 * PAD         # fine first-chunk: 1 group + taps
HALF = W // 2 + 2 * PAD    # half-row tile for batches 1..3
NWARM = 8                  # PE warm-up matmuls (bridge preamble->first data)


def _build_program():
    # Bacc (not plain Bass): its finalize() runs generate_event_semaphores,
    # which splits multi-sem waits into event-semaphore chains — the TRN2
    # walrus here accepts at most one sync wait per regular instruction.
    nc = bacc.Bacc()
    # x arrives host-padded: x[b, ci, :] = [0, 0, x_orig, 0, 0] (WPADDED cols)
    x_d = nc.declare_dram_parameter("x", [BPC, CIN, WPADDED], BF16, isOutput=False)
    # batch 0's first group repacked head-blob-style: [ci, cic, col] gives
    # 2064 B contiguous per partition, one fat-line startup transfer instead
    # of two thin ones (thin 1 KB lines supply only ~130 GB/s)
    x0h_d = nc.declare_dram_parameter("x0h", [P, N_CIC, NTF], BF16, isOutput=False)
    # weights host-packed as wt[ci, coc, k, cic, co]: each coc half is one
    # contiguous-line (2560 B/partition) transfer
    wt_d = nc.declare_dram_parameter(
        "wt", [P, N_COC, K, N_CIC, P], BF16, isOutput=False
    )
    b_d = nc.declare_dram_parameter("bias2", [P, N_COC], F32, isOutput=False)
    o_d = nc.declare_dram_parameter("out", [BPC, COUT, W], F32, isOutput=True)

    with tile.TileContext(nc) as tc:
        with (
            tc.tile_pool(name="wpool", bufs=1) as wpool,
            tc.tile_pool(name="xpool", bufs=4) as xpool,
            tc.tile_pool(name="opool", bufs=2 * N_COC) as opool,
            tc.tile_pool(name="psum", bufs=8, space="PSUM") as pspool,
        ):
            warm = wpool.tile([P, NT], BF16)
            nc.vector.memset(warm[:], 0.0)

            # Startup-critical loads, Sync trigger queue, consumption order.
            w_co = []
            for coc in range(N_COC):
                w_co.append(wpool.tile([P, K, N_CIC, P], BF16, name=f"w_co{coc}"))
            x0h = xpool.tile([P, N_CIC, NTF], BF16, tag="x0h", bufs=1)
            xf1 = []
            for cic in range(N_CIC):
                xf1.append(xpool.tile(
                    [P, NTF], BF16, tag="xf", bufs=2, name=f"xf1_{cic}"
                ))
            nc.sync.dma_start(w_co[0][:], wt_d[:, 0])
            nc.sync.dma_start(x0h[:], x0h_d[:])
            for cic in range(N_CIC):
                nc.sync.dma_start(
                    xf1[cic][:], x_d[0, cic * P:(cic + 1) * P, NT:NT + NTF]
                )

            # Rest of batch 0: 2-group halo chunks (chunk c covers padded
            # cols [1024c, 1024c + CWH), read by groups n=2c, 2c+1).
            # c1 loads immediately; c2/c3 are staged behind c1/c2's arrival
            # via gate dummies so the first-group-critical transfers get the
            # early bandwidth.
            scratch = wpool.tile([P, 1], BF16)
            xc = {}  # (c, cic) -> tile, c in 1..3
            for cic in range(N_CIC):
                t = xpool.tile([P, CWH], BF16, tag="xc1", bufs=N_CIC,
                               name=f"x0_1_{cic}")
                xc[(1, cic)] = t
                nc.sync.dma_start(
                    t[:], x_d[0, cic * P:(cic + 1) * P, 2 * NT:2 * NT + CWH]
                )
            b_sb = wpool.tile([P, N_COC], F32)
            nc.sync.dma_start(b_sb[:], b_d[:])
            # ring layout (bufs=4): [gc2 x2, gc3 x2, c2 x2 (alias gc2),
            # c3 x2 (alias gc3)] — each chunk's DMA waits only its gate,
            # whose writer waits the previous chunk's arrival
            gdum = {}
            for c in range(2, N_WT // 2):
                for cic in range(N_CIC):
                    gdum[(c, cic)] = xpool.tile(
                        [P, CWH], BF16, tag="xc", bufs=4, name=f"gc{c}_{cic}"
                    )
            for c in range(2, N_WT // 2):
                gate_src = xc[(c - 1, N_CIC - 1)]
                for cic in range(N_CIC):
                    dummy = gdum[(c, cic)]
                    nc.gpsimd.tensor_scalar_add(
                        dummy[:, 0:1], gate_src[:, 0:1], 0.0
                    )
                    nc.gpsimd.tensor_scalar_add(scratch[:], dummy[:, 0:1], 0.0)
                for cic in range(N_CIC):
                    t = xpool.tile([P, CWH], BF16, tag="xc", bufs=4,
                                   name=f"x0_{c}_{cic}")
                    xc[(c, cic)] = t
                    nc.sync.dma_start(
                        t[:],
                        x_d[0, cic * P:(cic + 1) * P,
                            c * 2 * NT:c * 2 * NT + CWH],
                    )
            # second weight half: trigger sits after c3's in the Sync queue,
            # keeping it out of the startup-critical window (needed ~17 us
            # later than w_co[0])
            nc.sync.dma_start(w_co[1][:], wt_d[:, 1])

            # Batches 1..3: half-row tiles deferred by gate aliasing. The 4
            # gate dummies are written (gpsimd) only once batch 0's last
            # chunk has landed; batch b+1's tiles alias batch b's (same ring
            # position), so each load starts once its predecessor half is
            # fully consumed — mid-way through the previous batch.
            last_c3 = xc[(N_WT // 2 - 1, N_CIC - 1)]
            for i in range(4):
                dummy = xpool.tile([P, HALF], BF16, tag="xh", bufs=4,
                                   name=f"gate{i}")
                nc.gpsimd.tensor_scalar_add(dummy[:, 0:1], last_c3[:, 0:1], 0.0)
                nc.gpsimd.tensor_scalar_add(scratch[:], dummy[:, 0:1], 0.0)
            xh = {}  # (b, h, cic) -> tile; h half covers cols [h*2048, +HALF)
            for b in range(1, BPC):
                for h in range(2):
                    for cic in range(N_CIC):
                        t = xpool.tile([P, HALF], BF16, tag="xh", bufs=4,
                                       name=f"x{b}_{h}_{cic}")
                        xh[(b, h, cic)] = t
                        nc.sync.dma_start(
                            t[:],
                            x_d[b, cic * P:(cic + 1) * P,
                                h * (W // 2):h * (W // 2) + HALF],
                        )

            ps_warm = pspool.tile([P, NT], F32, tag="ps", name="ps_warm")
            for _ in range(NWARM):
                nc.tensor.matmul(ps_warm[:], warm[:, 0:P], warm[:])

            for b in range(BPC):
                last_pass = b == BPC - 1
                for coc in range(N_COC):
                    last_coc = last_pass and coc == N_COC - 1
                    ot = opool.tile([P, W], F32, tag="o")
                    st = NT if last_coc else ST
                    for n in range(N_WT):
                        ps = pspool.tile([P, NT], F32, tag="ps", name=f"ps{b}_{coc}_{n}")
                        idx = 0
                        # cic-major: the first K matmuls of batch 0's first
                        # group need only the cic0 tile, giving cic1's DMA
                        # an extra ~1 us — a sub-300ns stall here resets the
                        # PE pstate ramp and costs ~2.5 us of half-speed
                        for cic in range(N_CIC):
                            for k in range(K):
                                if b == 0:
                                    if n == 0:
                                        rhs = x0h[:, cic, k:k + NT]
                                    elif n == 1:
                                        rhs = xf1[cic][:, k:k + NT]
                                    else:
                                        rhs = xc[(n // 2, cic)][
                                            :, (n % 2) * NT + k:(n % 2) * NT + k + NT
                                        ]
                                else:
                                    h, nh = divmod(n, N_WT // 2)
                                    rhs = xh[(b, h, cic)][
                                        :, nh * NT + k:nh * NT + k + NT
                                    ]
                                nc.tensor.matmul(
                                    ps[:],
                                    w_co[coc][:, k, cic, :],
                                    rhs,
                                    start=(idx == 0),
                                    stop=(idx == K * N_CIC - 1),
                                )
                                idx += 1
                        if last_coc and n == N_WT - 1:
                            # final group: drain the two 256-col halves on
                            # DIFFERENT engines (vector + scalar activation
                            # Identity = in*1 + bias) and trigger the stores
                            # from different queues, so the kernel tail is
                            # one quarter-size drain + one small store deep
                            lo = n * NT
                            hf = NT // 2
                            nc.vector.tensor_scalar_add(
                                ot[:, lo:lo + hf], ps[:, 0:hf],
                                b_sb[:, coc:coc + 1],
                            )
                            nc.sync.dma_start(
                                o_d[b, coc * P:(coc + 1) * P, lo:lo + hf],
                                ot[:, lo:lo + hf],
                            )
                            nc.scalar.activation(
                                ot[:, lo + hf:lo + NT], ps[:, hf:NT],
                                mybir.ActivationFunctionType.Identity,
                                bias=b_sb[:, coc:coc + 1],
                            )
                            nc.scalar.dma_start(
                                o_d[b, coc * P:(coc + 1) * P,
                                    lo + hf:lo + NT],
                                ot[:, lo + hf:lo + NT],
                            )
                        else:
                            nc.vector.tensor_scalar_add(
                                ot[:, n * NT:(n + 1) * NT], ps[:],
                                b_sb[:, coc:coc + 1],
                            )
                            # store as soon as a full chunk of st cols is
                            # ready (trigger from the Scalar engine; Sync
                            # paces the x loads)
                            if ((n + 1) * NT) % st == 0:
                                c0 = (n + 1) * NT - st
                                nc.scalar.dma_start(
                                    o_d[b, coc * P:(coc + 1) * P, c0:c0 + st],
                                    ot[:, c0:c0 + st],
                                )
    nc.finalize()
    return nc


_NC_CACHE = []


def kernel(x, weight, bias):
    assert x.shape == (B, CIN, W) and weight.shape == (COUT, CIN, K)
    if not _NC_CACHE:
        _NC_CACHE.append(_build_program())
    nc = _NC_CACHE[0]

    # wt[ci, coc, k, cic, co] = weight[coc*128+co, cic*128+ci, k]
    wt = np.ascontiguousarray(
        weight.astype(np.float32)
        .reshape(N_COC, P, N_CIC, P, K)   # [coc, co, cic, ci, k]
        .transpose(3, 0, 4, 2, 1)         # [ci, coc, k, cic, co]
    ).astype(ml_dtypes.bfloat16)
    bias2 = np.ascontiguousarray(bias.astype(np.float32).reshape(N_COC, P).T)
    xpad = np.pad(x.astype(np.float32), ((0, 0), (0, 0), (PAD, PAD))).astype(
        ml_dtypes.bfloat16
    )
    in_maps = [
        {
            "x": np.ascontiguousarray(xpad[i * BPC:(i + 1) * BPC]),
            # head blob: x0h[ci, cic, col] = xpad[first batch, cic*128+ci, col]
            "x0h": np.ascontiguousarray(
                xpad[i * BPC, :, :NTF].reshape(N_CIC, P, NTF).transpose(1, 0, 2)
            ),
            "wt": wt,
            "bias2": bias2,
        }
        for i in range(NCORES)
    ]
    res = run_bass_kernel_spmd(
        nc,
        in_maps,
        list(range(NCORES)),
        trace=bool(int(os.environ.get("KERNEL_TRACE", "0"))),
    )
    kernel.last_results = res
    return np.concatenate([res.results[i]["out"] for i in range(NCORES)], axis=0)
